# revision 53
# baseline (speedup 1.0000x reference)
"""Multi-head causal attention (B=2, S=2048, D=1024, H=16, Dh=64) on 8
axon-tunneled TRN2 NeuronCores.

Sharding: core = b*4 + g handles batch b and head group g (4 heads, 256
feature columns of the QKV projections / 256 rows of Wo).  Each core is
fully independent; the host sums the 4 per-head-group partial outputs of
each batch and adds the constant row bv @ Wo (softmax rows sum to 1, so
the V bias contributes a data-independent vector to every output row).

All matmul operands are bf16 (1 PE cycle/row vs 4 for fp32); PSUM
accumulation and the softmax denominator path stay fp32.

Per-core layout ("feature on partitions, seq on free"):
  xT   (1024, 2048)  = x[b].T                         bf16
  QT   (256, 2048)   = (0.125*Wq_g).T @ x.T + 0.125*bq_g   (scale folded on host)
  KT   (256, 2048)   = Wk_g.T @ x.T + bk_g
  va   (2048, 4, 65) = per-head [V_h | 1]  (ones col -> denominator row)
  S^T tiles (128k, 512q) = KT_h[:, kblk].T @ QT_h[:, qchunk]  (contraction 64)
  causal mask of the diagonal 128x128 chunk added on DVE
  P^T  = exp(S^T)  (no max-subtraction: |S| < 3)    bf16
  ctx_aug^T (65, 512q) = sum_k va_h[kblk].T @ P^T   (PSUM accumulate)
     rows 0:64 = unnormalized ctx^T, row 64 = softmax denominators
  recip: exp(-ln(denom)) on ACT [1,512], broadcast to 64 partitions on
     GPSIMD (partition_broadcast), ctxT = ctx_aug[0:64] * bcs on DVE
  out_partial (2048, 1024) = ctxT.T @ Wo_g

Scheduling: projections of q-chunk nq+1 are emitted between the
attention heads of q-chunk nq (the PE queue is in-order, so proj groups
back-fill PE while ACT runs softmax); within a head, scores for block
kb+2 are emitted before the PV matmul of block kb so the exp latency
hides under score streaming.
"""

import numpy as np

D_IN = 1024
D_OUT = 1024
H = 16
DH = 64
B = 2
S = 2048
NCORES = 8
HG = 4            # heads per core
DG = HG * DH      # 256 feature cols per core
MASK_NEG = -1.0e4
USE_POOL_BCAST = False  # InstPartitionBroadcast fails walrus codegen here

_state = {}


def _patch_tile_drain():
    """This image's walrus rejects instructions carrying >2 sync waits
    ("Too many sync wait commands"); Tile's final drain waits on every
    outstanding proc.  Split the waits into single-wait SP nops."""
    import concourse.tile as tile
    from concourse import mybir
    from concourse.vector_clock import ScopedClock

    if getattr(tile.TileContext._drain_and_barrier, "_split_waits", False):
        return

    def _drain_and_barrier(self, tick_clock, wait_clock):
        nc = self.nc
        probe = nc.sync.nop()
        wait_clock.add_sem_waits(
            probe.ins, ScopedClock({None: tick_clock.global_clock})
        )
        si = probe.ins.sync_info
        waits = list(si.on_wait) if si and si.on_wait else []
        if len(waits) > 1:
            probe.ins.sync_info = mybir.SyncInfo(
                on_wait=[waits[0]], on_update=list(si.on_update or [])
            )
            for w in waits[1:]:
                extra = nc.sync.nop()
                extra.ins.sync_info = mybir.SyncInfo(on_wait=[w], on_update=[])
        nc.sync.drain()

        nc.all_engine_barrier()
        assert self.sems is not None
        popped = nc._tile_sem_poison_stack.pop()
        assert popped is self._sem_poison
        nc.clear_and_free_semaphores(list(self.sems.allocated().values()))
        nc.all_engine_barrier()

    _drain_and_barrier._split_waits = True
    tile.TileContext._drain_and_barrier = _drain_and_barrier


def _split_excess_waits(nc, maxw=1):
    """Walrus in this image rejects instructions with too many sync-wait
    commands.  Hoist excess waits onto InstNoOp carriers inserted right
    before the offending instruction on the same engine (engines are
    in-order, so this preserves semantics)."""
    from concourse import mybir

    f = nc.m.functions[0]
    for bb in f.blocks:
        insts = bb.instructions  # live list
        i = 0
        while i < len(insts):
            ins = insts[i]
            si = ins.sync_info
            waits = list(si.on_wait) if si and si.on_wait else []
            if len(waits) > maxw:
                excess, keep = waits[:-maxw], waits[-maxw:]
                nops = []
                for j in range(0, len(excess), maxw):
                    nop = mybir.InstNoOp(
                        name=f"I-waitnop-{nc.next_id()}", ins=[], outs=[]
                    )
                    nop.engine = ins.engine
                    nop.sync_info = mybir.SyncInfo(
                        on_wait=excess[j : j + maxw], on_update=[]
                    )
                    nops.append(nop)
                ins.sync_info = mybir.SyncInfo(
                    on_wait=keep, on_update=list(si.on_update or [])
                )
                insts[i:i] = nops
                i += len(nops)
            i += 1


def _build_nc():
    import concourse.bass as bass
    import concourse.tile as tile
    from concourse import mybir

    _patch_tile_drain()
    FP = mybir.dt.float32
    BF = mybir.dt.bfloat16
    Alu = mybir.AluOpType
    Act = mybir.ActivationFunctionType

    nc = bass.Bass("TRN2", target_bir_lowering=False, debug=False)
    d_xT = nc.dram_tensor("xT", [8, 128, S], BF, kind="ExternalInput").ap()
    d_wq = nc.dram_tensor("wq", [128, 8 * DG], BF, kind="ExternalInput").ap()
    d_wk = nc.dram_tensor("wk", [128, 8 * DG], BF, kind="ExternalInput").ap()
    d_wv = nc.dram_tensor("wv", [128, 8 * DG], BF, kind="ExternalInput").ap()
    d_wo = nc.dram_tensor("wo", [2, 128, D_OUT], BF, kind="ExternalInput").ap()
    d_bq = nc.dram_tensor("bq", [2, 128, 1], FP, kind="ExternalInput").ap()
    d_bk = nc.dram_tensor("bk", [2, 128, 1], FP, kind="ExternalInput").ap()
    d_mask = nc.dram_tensor("mask", [128, 512], FP, kind="ExternalInput").ap()
    d_ones = nc.dram_tensor("ones", [65, DH], mybir.dt.float32r, kind="ExternalInput").ap()
    d_out = nc.dram_tensor("out", [S, D_OUT], BF, kind="ExternalOutput").ap()

    with tile.TileContext(nc) as tc:
        from contextlib import ExitStack

        with ExitStack() as ctx:
            const = ctx.enter_context(tc.tile_pool(name="const", bufs=1))
            qkv = ctx.enter_context(tc.tile_pool(name="qkv", bufs=1))

            wq_sb = const.tile([128, 8 * DG], BF, tag="wq")
            wk_sb = const.tile([128, 8 * DG], BF, tag="wk")
            wv_sb = const.tile([128, 8 * DG], BF, tag="wv")
            wo_sb = [const.tile([128, D_OUT], BF, tag=f"wo{i}", name=f"wo{i}") for i in range(2)]
            bq_sb = [const.tile([128, 1], FP, tag=f"bq{i}", name=f"bq{i}") for i in range(2)]
            bk_sb = [const.tile([128, 1], FP, tag=f"bk{i}", name=f"bk{i}") for i in range(2)]
            # mask_sb[:, 128j:128j+128] = triangle for diagonal block j of a
            # q-chunk: [k, q'] = -1e4 if q' < k else 0 (q' local to chunk)
            mask_sb = const.tile([128, 512], FP, tag="mask")
            # ones row for the denominator-recip broadcast matmul, at
            # partition 64 (same base as the denominator row)
            ones_sb = const.tile([65, DH], mybir.dt.float32r, tag="ones")

            xsb = [qkv.tile([128, S], BF, tag=f"x{i}", name=f"x{i}") for i in range(8)]
            qT = [qkv.tile([128, S], BF, tag=f"qT{i}", name=f"qT{i}") for i in range(2)]
            kT = [qkv.tile([128, S], BF, tag=f"kT{i}", name=f"kT{i}") for i in range(2)]
            va = [qkv.tile([128, HG, 65], BF, tag=f"va{i}", name=f"va{i}") for i in range(16)]
            ctxT = [qkv.tile([128, S], BF, tag=f"ctxT{i}", name=f"ctxT{i}") for i in range(2)]

            # input DMAs, ordered by first use.  x chunks are split at
            # column 512 so everything q-chunk 0 needs (wq/wk/wv + the
            # first 512 seq columns of x, ~2.5MB) lands before the bulk.
            nc.sync.dma_start(wq_sb[:, 0:1024], d_wq[:, 0:1024])
            nc.sync.dma_start(wq_sb[:, 1024:2048], d_wq[:, 1024:2048])
            for i in range(0, 8, 2):
                nc.sync.dma_start(xsb[i][:, 0:512], d_xT[i][:, 0:512])
            for i in range(1, 8, 2):
                nc.gpsimd.dma_start(xsb[i][:, 0:512], d_xT[i][:, 0:512])
            nc.sync.dma_start(wk_sb[:], d_wk)
            nc.gpsimd.dma_start(wv_sb[:], d_wv)
            for i in range(2):
                nc.sync.dma_start(bq_sb[i][:], d_bq[i])
                nc.sync.dma_start(bk_sb[i][:], d_bk[i])
            nc.sync.dma_start(mask_sb[:], d_mask)
            nc.sync.dma_start(ones_sb[:], d_ones)
            for i in range(8):
                eng = nc.sync if i % 2 == 0 else nc.gpsimd
                eng.dma_start(xsb[i][:, 512:1024], d_xT[i][:, 512:1024])
            for i in range(2):
                nc.gpsimd.dma_start(wo_sb[i][:], d_wo[i])
            for i in range(8):
                eng = nc.sync if i % 2 == 0 else nc.gpsimd
                eng.dma_start(xsb[i][:, 1024:S], d_xT[i][:, 1024:S])

            for st in range(16):
                nc.vector.memset(va[st][:, :, 64:65], 1.0)

            if USE_POOL_BCAST:
                from concourse import library_config

                nc.gpsimd.load_library(library_config.attn)

            with tc.tile_pool(name="pt", bufs=4) as ptp, tc.tile_pool(
                name="norm", bufs=3
            ) as normp, tc.tile_pool(name="osb", bufs=3) as osb, tc.tile_pool(
                name="mmpsum", bufs=2, space="PSUM"
            ) as pp, tc.tile_pool(
                name="spsum", bufs=3, space="PSUM"
            ) as sp, tc.tile_pool(
                name="cpsum", bufs=2, space="PSUM"
            ) as cp, tc.tile_pool(
                name="bpsum", bufs=1, space="PSUM"
            ) as bp:

                def proj_q(nq, m):
                    qs = slice(nq * 512, (nq + 1) * 512)
                    ms = m * 128
                    ps = pp.tile([128, 512], FP, tag="mm", name="ps")
                    for ci in range(8):
                        nc.tensor.matmul(
                            ps[:], wq_sb[:, ci * DG + ms : ci * DG + ms + 128],
                            xsb[ci][:, qs],
                            start=(ci == 0), stop=(ci == 7),
                        )
                    nc.vector.tensor_scalar(
                        qT[m][:, qs], ps[:], bq_sb[m][:], None, Alu.add
                    )

                def proj_k(nq, m):
                    qs = slice(nq * 512, (nq + 1) * 512)
                    ms = m * 128
                    ps2 = pp.tile([128, 512], FP, tag="mm", name="ps2")
                    for ci in range(8):
                        nc.tensor.matmul(
                            ps2[:], wk_sb[:, ci * DG + ms : ci * DG + ms + 128],
                            xsb[ci][:, qs],
                            start=(ci == 0), stop=(ci == 7),
                        )
                    nc.vector.tensor_scalar(
                        kT[m][:, qs], ps2[:], bk_sb[m][:], None, Alu.add
                    )

                def proj_v(st):
                    ss = slice(st * 128, (st + 1) * 128)
                    psv = pp.tile([128, 512], FP, tag="mm", name="psv")
                    for ci in range(8):
                        nc.tensor.matmul(
                            psv[:, 0:DG], xsb[ci][:, ss],
                            wv_sb[:, ci * DG : (ci + 1) * DG],
                            start=(ci == 0), stop=(ci == 7),
                        )
                    nc.vector.tensor_copy(
                        va[st][:, :, 0:64],
                        psv[:, 0:DG].rearrange("p (h d) -> p h d", h=HG),
                    )

                def attn_head(qc, h, pending_fin):
                    """Emit one head's attention.  `pending_fin` (the
                    previous head's normalization) is emitted after this
                    head's score prologue so its PE broadcast matmul never
                    stalls the in-order PE queue waiting on ACT's ln.
                    Returns this head's own finish closure."""
                    qs = slice(qc * 512, (qc + 1) * 512)
                    ht, hp = h // 2, (h % 2) * 64
                    hs = slice(hp, hp + 64)
                    nkb = 4 * qc + 4
                    cps = cp.tile([65, 512], FP, tag="ctx", name="cps")

                    def emit_scores(kb):
                        ks = slice(kb * 128, (kb + 1) * 128)
                        j = kb - 4 * qc
                        lo = 128 * j if j > 0 else 0
                        qsl = slice(qc * 512 + lo, (qc + 1) * 512)
                        sps = sp.tile([128, 512], FP, tag="s", name="sps")
                        nc.tensor.matmul(
                            sps[:, lo:], kT[ht][hs, ks], qT[ht][hs, qsl],
                            start=True, stop=True,
                        )
                        if j >= 0:
                            nc.vector.tensor_add(
                                sps[:, lo : lo + 128], sps[:, lo : lo + 128],
                                mask_sb[:, lo : lo + 128],
                            )
                        pt = ptp.tile([128, 512], BF, tag="pt", name="pt")
                        nc.scalar.activation(pt[:, lo:], sps[:, lo:], Act.Exp)
                        return pt, lo

                    def emit_pv(kb, pt, lo):
                        nc.tensor.matmul(
                            cps[:, lo:], va[kb][:, h, :], pt[:, lo:],
                            start=(kb == 0), stop=(kb == nkb - 1),
                        )

                    # scores run 2 blocks ahead of PV so the exp latency
                    # hides under PE score streaming (in-order PE queue)
                    pend = {}
                    for kb in range(min(2, nkb)):
                        pend[kb] = emit_scores(kb)
                    if pending_fin is not None:
                        pending_fin()
                    for kb in range(nkb):
                        if kb + 2 < nkb:
                            pend[kb + 2] = emit_scores(kb + 2)
                        emit_pv(kb, *pend.pop(kb))

                    def fin():
                        # normalization: ln(denom) at partition 64 (ACT),
                        # PE broadcast matmul to [64,512] (fp32r: fp32
                        # bits, 1 cycle/row), exp(-x) on ACT -> SBUF.
                        # (A DMA-broadcast variant was measured 35us
                        # slower; GPSIMD partition_broadcast fails codegen.)
                        rec0 = normp.tile([65, 512], FP, tag="rec0", name="rec0")
                        nc.scalar.activation(
                            rec0[64:65, :].bitcast(mybir.dt.float32r),
                            cps[64:65, :], Act.Ln,
                        )
                        bc = bp.tile([64, 512], FP, tag="bc", name="bc")
                        nc.tensor.matmul(
                            bc[:], ones_sb[64:65, :],
                            rec0[64:65, :].bitcast(mybir.dt.float32r),
                            start=True, stop=True,
                        )
                        bcs = normp.tile([64, 512], FP, tag="bcs", name="bcs")
                        nc.scalar.activation(bcs[:], bc[:], Act.Exp, scale=-1.0)
                        if hp == 0:
                            dst = ctxT[ht][hs, qs]
                        else:
                            tmp2 = normp.tile([64, 512], BF, tag="tmp2", name="tmp2")
                            dst = tmp2[:]
                        nc.vector.tensor_mul(dst, cps[0:64, :], bcs[:])
                        if hp != 0:
                            # DVE lanes cannot cross partitions; shift the
                            # odd head's rows to partitions 64..127 by DMA
                            nc.sync.dma_start(ctxT[ht][hs, qs], dst)

                    return fin

                def out_proj(st, tail=False):
                    ss = slice(st * 128, (st + 1) * 128)
                    for n in range(2):
                        ns = slice(n * 512, (n + 1) * 512)
                        po = pp.tile([128, 512], FP, tag="mm", name="po")
                        for cb in range(2):
                            nc.tensor.matmul(
                                po[:], ctxT[cb][:, ss], wo_sb[cb][:, ns],
                                start=(cb == 0), stop=(cb == 1),
                            )
                        ot = osb.tile([128, 512], BF, tag="ot", name="ot")
                        # in the final q-chunk ACT is idle: split the two
                        # copies of each s-tile across DVE and ACT so the
                        # tail drains in parallel
                        if tail and n == 1:
                            nc.scalar.copy(ot[:], po[:])
                        else:
                            nc.vector.tensor_copy(ot[:], po[:])
                        nc.gpsimd.dma_start(d_out[ss, ns], ot[:])

                # q-chunk 0 projections up front (DMA-bound startup)
                for m in range(2):
                    proj_q(0, m)
                for m in range(2):
                    proj_k(0, m)
                for st in range(4):
                    proj_v(st)

                # steady state: attention for qc interleaved with the
                # projections of chunk qc+1 (odd heads first so the odd
                # ctxT shift DMAs land before the output projection)
                for qc in range(4):
                    filler = []
                    if qc < 3:
                        nxt = qc + 1
                        filler = (
                            [lambda m=m: proj_q(nxt, m) for m in range(2)]
                            + [lambda m=m: proj_k(nxt, m) for m in range(2)]
                            + [lambda st=st: proj_v(st) for st in range(4 * nxt, 4 * nxt + 4)]
                        )
                    fills = [filler[0:2], filler[2:4], filler[4:6], filler[6:8]]
                    pending_fin = None
                    for idx, h in enumerate((1, 3, 0, 2)):
                        pending_fin = attn_head(qc, h, pending_fin)
                        for f in fills[idx]:
                            f()
                    pending_fin()
                    for st in range(4 * qc, 4 * qc + 4):
                        out_proj(st, tail=(qc == 3))

    _split_excess_waits(nc)
    return nc


def _get_nc():
    if "nc" not in _state:
        _state["nc"] = _build_nc()
    return _state["nc"]


def _build_in_maps(x, Wq, bq, Wk, bk, Wv, bv, Wo):
    import ml_dtypes

    bf = ml_dtypes.bfloat16
    x = np.asarray(x, np.float32)
    Wq = np.asarray(Wq, np.float32)
    bq = np.asarray(bq, np.float32)
    Wk = np.asarray(Wk, np.float32)
    bk = np.asarray(bk, np.float32)
    Wv = np.asarray(Wv, np.float32)
    bv = np.asarray(bv, np.float32)
    Wo = np.asarray(Wo, np.float32)

    # mask[k, 128j + r] = -1e4 if r < k else 0  (triangle for diag block j)
    k = np.arange(128)[:, None]
    r = np.arange(128)[None, :]
    tri = np.where(r < k, MASK_NEG, 0.0).astype(np.float32)
    mask = np.tile(tri, (1, 4)).astype(np.float32)

    def wchunks(W, scale=1.0):
        # [1024, 256] -> [128, 8*256] with contraction chunk-major cols
        return np.ascontiguousarray(
            (W * scale).reshape(8, 128, DG).transpose(1, 0, 2).reshape(128, 8 * DG)
        ).astype(bf)

    in_maps = []
    for core in range(NCORES):
        b, g = core // HG, core % HG
        cs = slice(g * DG, (g + 1) * DG)
        xT = np.ascontiguousarray(x[b].T).reshape(8, 128, S).astype(bf)
        in_maps.append(
            {
                "xT": xT,
                "wq": wchunks(Wq[:, cs], 0.125),
                "wk": wchunks(Wk[:, cs]),
                "wv": wchunks(Wv[:, cs]),
                "wo": np.ascontiguousarray(Wo[cs, :]).reshape(2, 128, D_OUT).astype(bf),
                "bq": (0.125 * bq[cs]).reshape(2, 128, 1).astype(np.float32),
                "bk": bk[cs].reshape(2, 128, 1).astype(np.float32),
                "mask": mask,
                "ones": np.ones((65, DH), np.float32),
            }
        )
    return in_maps


def kernel(x, Wq, bq, Wk, bk, Wv, bv, Wo):
    from concourse.bass_utils import run_bass_kernel_spmd

    nc = _get_nc()
    in_maps = _build_in_maps(x, Wq, bq, Wk, bk, Wv, bv, Wo)
    _state["in_maps"] = in_maps

    res = run_bass_kernel_spmd(nc, in_maps, list(range(NCORES)))
    out = np.zeros((B, S, D_OUT), np.float64)
    for core in range(NCORES):
        out[core // HG] += res.results[core]["out"].astype(np.float32)
    # V-bias contribution: attn rows sum to 1, so ctx = P@(xWv) + bv and
    # the output picks up the constant row bv @ Wo.
    out += (np.asarray(bv, np.float64) @ np.asarray(Wo, np.float64))[None, None, :]
    return out.astype(np.float32)


# revision 56
# speedup vs baseline: 1.0848x; 1.0848x over previous
"""Multi-head causal attention (B=2, S=2048, D=1024, H=16, Dh=64) on 8
axon-tunneled TRN2 NeuronCores.

Sharding: core = b*4 + g handles batch b and head group g (4 heads, 256
feature columns of the QKV projections / 256 rows of Wo).  Each core is
fully independent; the host sums the 4 per-head-group partial outputs of
each batch and adds the constant row bv @ Wo (softmax rows sum to 1, so
the V bias contributes a data-independent vector to every output row).

All matmul operands are bf16 (1 PE cycle/row vs 4 for fp32); PSUM
accumulation and the softmax denominator path stay fp32.

Per-core layout ("feature on partitions, seq on free"):
  xT   (1024, 2048)  = x[b].T                         bf16
  QT   (256, 2048)   = (0.125*Wq_g).T @ x.T + 0.125*bq_g   (scale folded on host)
  KT   (256, 2048)   = Wk_g.T @ x.T + bk_g
  va   (2048, 4, 65) = per-head [V_h | 1]  (ones col -> denominator row)
  S^T tiles (128k, 512q) = KT_h[:, kblk].T @ QT_h[:, qchunk]  (contraction 64)
  causal mask of the diagonal 128x128 chunk added on DVE
  P^T  = exp(S^T)  (no max-subtraction: |S| < 3)    bf16
  ctx_aug^T (65, 512q) = sum_k va_h[kblk].T @ P^T   (PSUM accumulate)
     rows 0:64 = unnormalized ctx^T, row 64 = softmax denominators
  recip: exp(-ln(denom)) on ACT [1,512], broadcast to 64 partitions on
     GPSIMD (partition_broadcast), ctxT = ctx_aug[0:64] * bcs on DVE
  out_partial (2048, 1024) = ctxT.T @ Wo_g

Scheduling: projections of q-chunk nq+1 are emitted between the
attention heads of q-chunk nq (the PE queue is in-order, so proj groups
back-fill PE while ACT runs softmax); within a head, scores for block
kb+2 are emitted before the PV matmul of block kb so the exp latency
hides under score streaming.
"""

import numpy as np

D_IN = 1024
D_OUT = 1024
H = 16
DH = 64
B = 2
S = 2048
NCORES = 8
HG = 4            # heads per core
DG = HG * DH      # 256 feature cols per core
MASK_NEG = -1.0e4
USE_POOL_BCAST = False  # InstPartitionBroadcast fails walrus codegen here

_state = {}


def _patch_tile_drain():
    """This image's walrus rejects instructions carrying >2 sync waits
    ("Too many sync wait commands"); Tile's final drain waits on every
    outstanding proc.  Split the waits into single-wait SP nops."""
    import concourse.tile as tile
    from concourse import mybir
    from concourse.vector_clock import ScopedClock

    if getattr(tile.TileContext._drain_and_barrier, "_split_waits", False):
        return

    def _drain_and_barrier(self, tick_clock, wait_clock):
        nc = self.nc
        probe = nc.sync.nop()
        wait_clock.add_sem_waits(
            probe.ins, ScopedClock({None: tick_clock.global_clock})
        )
        si = probe.ins.sync_info
        waits = list(si.on_wait) if si and si.on_wait else []
        if len(waits) > 1:
            probe.ins.sync_info = mybir.SyncInfo(
                on_wait=[waits[0]], on_update=list(si.on_update or [])
            )
            for w in waits[1:]:
                extra = nc.sync.nop()
                extra.ins.sync_info = mybir.SyncInfo(on_wait=[w], on_update=[])
        nc.sync.drain()

        nc.all_engine_barrier()
        assert self.sems is not None
        popped = nc._tile_sem_poison_stack.pop()
        assert popped is self._sem_poison
        nc.clear_and_free_semaphores(list(self.sems.allocated().values()))
        nc.all_engine_barrier()

    _drain_and_barrier._split_waits = True
    tile.TileContext._drain_and_barrier = _drain_and_barrier


def _split_excess_waits(nc, maxw=1):
    """Walrus in this image rejects instructions with too many sync-wait
    commands.  Hoist excess waits onto InstNoOp carriers inserted right
    before the offending instruction on the same engine (engines are
    in-order, so this preserves semantics)."""
    from concourse import mybir

    f = nc.m.functions[0]
    for bb in f.blocks:
        insts = bb.instructions  # live list
        i = 0
        while i < len(insts):
            ins = insts[i]
            si = ins.sync_info
            waits = list(si.on_wait) if si and si.on_wait else []
            if len(waits) > maxw:
                excess, keep = waits[:-maxw], waits[-maxw:]
                nops = []
                for j in range(0, len(excess), maxw):
                    nop = mybir.InstNoOp(
                        name=f"I-waitnop-{nc.next_id()}", ins=[], outs=[]
                    )
                    nop.engine = ins.engine
                    nop.sync_info = mybir.SyncInfo(
                        on_wait=excess[j : j + maxw], on_update=[]
                    )
                    nops.append(nop)
                ins.sync_info = mybir.SyncInfo(
                    on_wait=keep, on_update=list(si.on_update or [])
                )
                insts[i:i] = nops
                i += len(nops)
            i += 1


def _build_nc():
    import concourse.bass as bass
    import concourse.tile as tile
    from concourse import mybir

    _patch_tile_drain()
    FP = mybir.dt.float32
    BF = mybir.dt.bfloat16
    Alu = mybir.AluOpType
    Act = mybir.ActivationFunctionType

    nc = bass.Bass("TRN2", target_bir_lowering=False, debug=False)
    d_xT = nc.dram_tensor("xT", [8, 128, S], BF, kind="ExternalInput").ap()
    d_wq = nc.dram_tensor("wq", [128, 8 * DG], BF, kind="ExternalInput").ap()
    d_wk = nc.dram_tensor("wk", [128, 8 * DG], BF, kind="ExternalInput").ap()
    d_wv = nc.dram_tensor("wv", [128, 8 * DG], BF, kind="ExternalInput").ap()
    d_wo = nc.dram_tensor("wo", [2, 128, D_OUT], BF, kind="ExternalInput").ap()
    d_bq = nc.dram_tensor("bq", [2, 128, 1], FP, kind="ExternalInput").ap()
    d_bk = nc.dram_tensor("bk", [2, 128, 1], FP, kind="ExternalInput").ap()
    d_mask = nc.dram_tensor("mask", [128, 512], FP, kind="ExternalInput").ap()
    d_ones = nc.dram_tensor("ones", [65, DH], mybir.dt.float32r, kind="ExternalInput").ap()
    d_out = nc.dram_tensor("out", [S, D_OUT], BF, kind="ExternalOutput").ap()

    with tile.TileContext(nc) as tc:
        from contextlib import ExitStack

        with ExitStack() as ctx:
            const = ctx.enter_context(tc.tile_pool(name="const", bufs=1))
            qkv = ctx.enter_context(tc.tile_pool(name="qkv", bufs=1))

            wq_sb = const.tile([128, 8 * DG], BF, tag="wq")
            wk_sb = const.tile([128, 8 * DG], BF, tag="wk")
            wv_sb = const.tile([128, 8 * DG], BF, tag="wv")
            wo_sb = [const.tile([128, D_OUT], BF, tag=f"wo{i}", name=f"wo{i}") for i in range(2)]
            bq_sb = [const.tile([128, 1], FP, tag=f"bq{i}", name=f"bq{i}") for i in range(2)]
            bk_sb = [const.tile([128, 1], FP, tag=f"bk{i}", name=f"bk{i}") for i in range(2)]
            # mask_sb[:, 128j:128j+128] = triangle for diagonal block j of a
            # q-chunk: [k, q'] = -1e4 if q' < k else 0 (q' local to chunk)
            mask_sb = const.tile([128, 512], FP, tag="mask")
            # ones row for the denominator-recip broadcast matmul, at
            # partition 64 (same base as the denominator row)
            ones_sb = const.tile([65, DH], mybir.dt.float32r, tag="ones")

            xsb = [qkv.tile([128, S], BF, tag=f"x{i}", name=f"x{i}") for i in range(8)]
            qT = [qkv.tile([128, S], BF, tag=f"qT{i}", name=f"qT{i}") for i in range(2)]
            kT = [qkv.tile([128, S], BF, tag=f"kT{i}", name=f"kT{i}") for i in range(2)]
            va = [qkv.tile([128, HG, 65], BF, tag=f"va{i}", name=f"va{i}") for i in range(16)]
            ctxT = [qkv.tile([128, S], BF, tag=f"ctxT{i}", name=f"ctxT{i}") for i in range(2)]

            # input DMAs, ordered by first use.  x chunks are split at
            # column 512 so everything q-chunk 0 needs (wq/wk/wv + the
            # first 512 seq columns of x, ~2.5MB) lands before the bulk.
            nc.sync.dma_start(wq_sb[:], d_wq)
            for i in range(0, 8, 2):
                nc.sync.dma_start(xsb[i][:, 0:512], d_xT[i][:, 0:512])
            for i in range(1, 8, 2):
                nc.gpsimd.dma_start(xsb[i][:, 0:512], d_xT[i][:, 0:512])
            nc.sync.dma_start(wk_sb[:], d_wk)
            nc.gpsimd.dma_start(wv_sb[:], d_wv)
            for i in range(2):
                nc.sync.dma_start(bq_sb[i][:], d_bq[i])
                nc.sync.dma_start(bk_sb[i][:], d_bk[i])
            nc.sync.dma_start(mask_sb[:], d_mask)
            nc.sync.dma_start(ones_sb[:], d_ones)
            for i in range(8):
                eng = nc.sync if i % 2 == 0 else nc.gpsimd
                eng.dma_start(xsb[i][:, 512:1024], d_xT[i][:, 512:1024])
            for i in range(2):
                nc.gpsimd.dma_start(wo_sb[i][:], d_wo[i])
            for i in range(8):
                eng = nc.sync if i % 2 == 0 else nc.gpsimd
                eng.dma_start(xsb[i][:, 1024:S], d_xT[i][:, 1024:S])

            for st in range(16):
                nc.vector.memset(va[st][:, :, 64:65], 1.0)

            if USE_POOL_BCAST:
                from concourse import library_config

                nc.gpsimd.load_library(library_config.attn)

            with tc.tile_pool(name="pt", bufs=4) as ptp, tc.tile_pool(
                name="norm", bufs=3
            ) as normp, tc.tile_pool(name="osb", bufs=3) as osb, tc.tile_pool(
                name="mmpsum", bufs=2, space="PSUM"
            ) as pp, tc.tile_pool(
                name="spsum", bufs=3, space="PSUM"
            ) as sp, tc.tile_pool(
                name="cpsum", bufs=2, space="PSUM"
            ) as cp, tc.tile_pool(
                name="bpsum", bufs=1, space="PSUM"
            ) as bp:

                def proj_q(nq, m):
                    qs = slice(nq * 512, (nq + 1) * 512)
                    ms = m * 128
                    ps = pp.tile([128, 512], FP, tag="mm", name="ps")
                    for ci in range(8):
                        nc.tensor.matmul(
                            ps[:], wq_sb[:, ci * DG + ms : ci * DG + ms + 128],
                            xsb[ci][:, qs],
                            start=(ci == 0), stop=(ci == 7),
                        )
                    nc.vector.tensor_scalar(
                        qT[m][:, qs], ps[:], bq_sb[m][:], None, Alu.add
                    )

                def proj_k(nq, m):
                    qs = slice(nq * 512, (nq + 1) * 512)
                    ms = m * 128
                    ps2 = pp.tile([128, 512], FP, tag="mm", name="ps2")
                    for ci in range(8):
                        nc.tensor.matmul(
                            ps2[:], wk_sb[:, ci * DG + ms : ci * DG + ms + 128],
                            xsb[ci][:, qs],
                            start=(ci == 0), stop=(ci == 7),
                        )
                    nc.vector.tensor_scalar(
                        kT[m][:, qs], ps2[:], bk_sb[m][:], None, Alu.add
                    )

                def proj_v(st):
                    ss = slice(st * 128, (st + 1) * 128)
                    psv = pp.tile([128, 512], FP, tag="mm", name="psv")
                    for ci in range(8):
                        nc.tensor.matmul(
                            psv[:, 0:DG], xsb[ci][:, ss],
                            wv_sb[:, ci * DG : (ci + 1) * DG],
                            start=(ci == 0), stop=(ci == 7),
                        )
                    nc.vector.tensor_copy(
                        va[st][:, :, 0:64],
                        psv[:, 0:DG].rearrange("p (h d) -> p h d", h=HG),
                    )

                def attn_head(qc, h, pending_fin):
                    """Emit one head's attention.  `pending_fin` (the
                    previous head's normalization) is emitted after this
                    head's score prologue so its PE broadcast matmul never
                    stalls the in-order PE queue waiting on ACT's ln.
                    Returns this head's own finish closure."""
                    qs = slice(qc * 512, (qc + 1) * 512)
                    ht, hp = h // 2, (h % 2) * 64
                    hs = slice(hp, hp + 64)
                    nkb = 4 * qc + 4
                    cps = cp.tile([65, 512], FP, tag="ctx", name="cps")

                    def emit_scores(kb):
                        ks = slice(kb * 128, (kb + 1) * 128)
                        j = kb - 4 * qc
                        lo = 128 * j if j > 0 else 0
                        qsl = slice(qc * 512 + lo, (qc + 1) * 512)
                        sps = sp.tile([128, 512], FP, tag="s", name="sps")
                        nc.tensor.matmul(
                            sps[:, lo:], kT[ht][hs, ks], qT[ht][hs, qsl],
                            start=True, stop=True,
                        )
                        if j >= 0:
                            nc.vector.tensor_add(
                                sps[:, lo : lo + 128], sps[:, lo : lo + 128],
                                mask_sb[:, lo : lo + 128],
                            )
                        pt = ptp.tile([128, 512], BF, tag="pt", name="pt")
                        nc.scalar.activation(pt[:, lo:], sps[:, lo:], Act.Exp)
                        return pt, lo

                    def emit_pv(kb, pt, lo):
                        nc.tensor.matmul(
                            cps[:, lo:], va[kb][:, h, :], pt[:, lo:],
                            start=(kb == 0), stop=(kb == nkb - 1),
                        )

                    # scores run 2 blocks ahead of PV so the exp latency
                    # hides under PE score streaming (in-order PE queue)
                    pend = {}
                    for kb in range(min(2, nkb)):
                        pend[kb] = emit_scores(kb)
                    if pending_fin is not None:
                        pending_fin()
                    for kb in range(nkb):
                        if kb + 2 < nkb:
                            pend[kb + 2] = emit_scores(kb + 2)
                        emit_pv(kb, *pend.pop(kb))

                    def fin():
                        # normalization: ln(denom) at partition 64 (ACT),
                        # PE broadcast matmul to [64,512] (fp32r: fp32
                        # bits, 1 cycle/row), exp(-x) on ACT -> SBUF.
                        # (A DMA-broadcast variant was measured 35us
                        # slower; GPSIMD partition_broadcast fails codegen.)
                        rec0 = normp.tile([65, 512], FP, tag="rec0", name="rec0")
                        nc.scalar.activation(
                            rec0[64:65, :].bitcast(mybir.dt.float32r),
                            cps[64:65, :], Act.Ln,
                        )
                        bc = bp.tile([64, 512], FP, tag="bc", name="bc")
                        nc.tensor.matmul(
                            bc[:], ones_sb[64:65, :],
                            rec0[64:65, :].bitcast(mybir.dt.float32r),
                            start=True, stop=True,
                        )
                        bcs = normp.tile([64, 512], FP, tag="bcs", name="bcs")
                        nc.scalar.activation(bcs[:], bc[:], Act.Exp, scale=-1.0)
                        if hp == 0:
                            dst = ctxT[ht][hs, qs]
                        else:
                            tmp2 = normp.tile([64, 512], BF, tag="tmp2", name="tmp2")
                            dst = tmp2[:]
                        nc.vector.tensor_mul(dst, cps[0:64, :], bcs[:])
                        if hp != 0:
                            # DVE lanes cannot cross partitions; shift the
                            # odd head's rows to partitions 64..127 by DMA
                            nc.sync.dma_start(ctxT[ht][hs, qs], dst)

                    return fin

                def out_proj(st):
                    ss = slice(st * 128, (st + 1) * 128)
                    for n in range(2):
                        ns = slice(n * 512, (n + 1) * 512)
                        po = pp.tile([128, 512], FP, tag="mm", name="po")
                        for cb in range(2):
                            nc.tensor.matmul(
                                po[:], ctxT[cb][:, ss], wo_sb[cb][:, ns],
                                start=(cb == 0), stop=(cb == 1),
                            )
                        ot = osb.tile([128, 512], BF, tag="ot", name="ot")
                        nc.vector.tensor_copy(ot[:], po[:])
                        nc.gpsimd.dma_start(d_out[ss, ns], ot[:])

                # q-chunk 0 projections up front (DMA-bound startup)
                for m in range(2):
                    proj_q(0, m)
                for m in range(2):
                    proj_k(0, m)
                for st in range(4):
                    proj_v(st)

                # steady state: attention for qc interleaved with the
                # projections of chunk qc+1 (odd heads first so the odd
                # ctxT shift DMAs land before the output projection)
                for qc in range(4):
                    filler = []
                    if qc < 3:
                        nxt = qc + 1
                        filler = (
                            [lambda m=m: proj_q(nxt, m) for m in range(2)]
                            + [lambda m=m: proj_k(nxt, m) for m in range(2)]
                            + [lambda st=st: proj_v(st) for st in range(4 * nxt, 4 * nxt + 4)]
                        )
                    fills = [filler[0:2], filler[2:4], filler[4:6], filler[6:8]]
                    pending_fin = None
                    for idx, h in enumerate((1, 3, 0, 2)):
                        pending_fin = attn_head(qc, h, pending_fin)
                        for f in fills[idx]:
                            f()
                    pending_fin()
                    for st in range(4 * qc, 4 * qc + 4):
                        out_proj(st)

    _split_excess_waits(nc)
    return nc


def _get_nc():
    if "nc" not in _state:
        _state["nc"] = _build_nc()
    return _state["nc"]


def _build_in_maps(x, Wq, bq, Wk, bk, Wv, bv, Wo):
    import ml_dtypes

    bf = ml_dtypes.bfloat16
    x = np.asarray(x, np.float32)
    Wq = np.asarray(Wq, np.float32)
    bq = np.asarray(bq, np.float32)
    Wk = np.asarray(Wk, np.float32)
    bk = np.asarray(bk, np.float32)
    Wv = np.asarray(Wv, np.float32)
    bv = np.asarray(bv, np.float32)
    Wo = np.asarray(Wo, np.float32)

    # mask[k, 128j + r] = -1e4 if r < k else 0  (triangle for diag block j)
    k = np.arange(128)[:, None]
    r = np.arange(128)[None, :]
    tri = np.where(r < k, MASK_NEG, 0.0).astype(np.float32)
    mask = np.tile(tri, (1, 4)).astype(np.float32)

    def wchunks(W, scale=1.0):
        # [1024, 256] -> [128, 8*256] with contraction chunk-major cols
        return np.ascontiguousarray(
            (W * scale).reshape(8, 128, DG).transpose(1, 0, 2).reshape(128, 8 * DG)
        ).astype(bf)

    in_maps = []
    for core in range(NCORES):
        b, g = core // HG, core % HG
        cs = slice(g * DG, (g + 1) * DG)
        xT = np.ascontiguousarray(x[b].T).reshape(8, 128, S).astype(bf)
        in_maps.append(
            {
                "xT": xT,
                "wq": wchunks(Wq[:, cs], 0.125),
                "wk": wchunks(Wk[:, cs]),
                "wv": wchunks(Wv[:, cs]),
                "wo": np.ascontiguousarray(Wo[cs, :]).reshape(2, 128, D_OUT).astype(bf),
                "bq": (0.125 * bq[cs]).reshape(2, 128, 1).astype(np.float32),
                "bk": bk[cs].reshape(2, 128, 1).astype(np.float32),
                "mask": mask,
                "ones": np.ones((65, DH), np.float32),
            }
        )
    return in_maps


def kernel(x, Wq, bq, Wk, bk, Wv, bv, Wo):
    from concourse.bass_utils import run_bass_kernel_spmd

    nc = _get_nc()
    in_maps = _build_in_maps(x, Wq, bq, Wk, bk, Wv, bv, Wo)
    _state["in_maps"] = in_maps

    res = run_bass_kernel_spmd(nc, in_maps, list(range(NCORES)))
    out = np.zeros((B, S, D_OUT), np.float64)
    for core in range(NCORES):
        out[core // HG] += res.results[core]["out"].astype(np.float32)
    # V-bias contribution: attn rows sum to 1, so ctx = P@(xWv) + bv and
    # the output picks up the constant row bv @ Wo.
    out += (np.asarray(bv, np.float64) @ np.asarray(Wo, np.float64))[None, None, :]
    return out.astype(np.float32)
